# revision 24
# baseline (speedup 1.0000x reference)
"""Trainium2 Bass kernel for nn_DensityEdgeProjection.

Strategy (sharding_hint): the ns*nb*nb density-element (KV token) axis is
sharded over 8 cores as 768 (spin, i) rows of 384 j-tokens each -> 96 rows
per core.  Each core encodes its token slice and produces partial attention
numerators/denominators (no max -- scores are provably tiny), which the host
combines (flash-attention style) and pushes through the small output MLP.

Key algebraic restructure vs the v1 kernel: encoder layer 3 is linear
(no activation before the layernorm), so its weight w3 is folded on the
host into the attention projections (W_SV = w3p @ [WQ | wv_p]) and the
layernorm statistic E[x^2] is computed as the quadratic form
that^T (w3p w3p^T/D) that.  This removes the L3 matmul output
materialization (PSUM->SBUF copy) and the x^2 squaring pass.

Device layout: activations are feature-major (features on partitions,
tokens on the free axis).  All matmuls bf16 with fp32 PSUM accumulation.
S and V projections are fused into one 512-wide matmul per token block
(exactly one PSUM bank).
"""

import os
import sys
import numpy as np

sys.path.insert(0, "/opt/trn_rl_repo")

from ml_dtypes import bfloat16

NB = 384
NS = 2
D = 256
TQ = 32
H = 8
DH = 32
MAX_L = 2
NCORES = 8
NROW_TOTAL = NS * NB          # 768 (s, i) rows
NROW = NROW_TOTAL // NCORES   # 96 rows per core
EPS = 1e-5

LAST_EXEC_NS = None
LAST_RESULTS = None

_PROGRAM = None


def _np_silu(x):
    return x / (1.0 + np.exp(-x))


def _np_layernorm(x, w, b):
    mu = x.mean(-1, keepdims=True)
    var = x.var(-1, keepdims=True)
    return (x - mu) / np.sqrt(var + EPS) * w + b


S3 = 16.0   # fp8 range scale for the folded w3 projection


def _blob_layout(nrow):
    """element offsets (bf16) for each packed section, 512-aligned."""
    sections = [
        ("rho", (nrow, 3, NB)),
        ("lA", (nrow, 3, D)),
        ("ajT", (2, 128, NB)),
        ("w2", (128, 512)),
        ("g", (128, 512)),
        ("wsv", (128, 1024)),
        ("onesw", (128, 1)),
        ("ident", (128, 128)),
    ]
    offs = {}
    off = 0
    for k, shp in sections:
        n = int(np.prod(shp))
        offs[k] = (off, n, shp)
        off += (n + 511) // 512 * 512
    return offs, off


def _pack_blob(parts, nrow):
    offs, total = _blob_layout(nrow)
    blob = np.zeros(total, bfloat16)
    for k, arr in parts.items():
        off, n, shp = offs[k]
        assert tuple(arr.shape) == tuple(shp), (k, arr.shape, shp)
        blob[off:off + n] = arr.astype(bfloat16).ravel()
    return blob


def _build_program(nrow=NROW, repeat=1):
    """Build the (single, shared across cores) Bass/Tile program."""
    NROW_L = nrow
    import concourse.bass as bass
    import concourse.bacc as bacc
    import concourse.tile as tile
    from concourse import mybir

    f32 = mybir.dt.float32
    bf16 = mybir.dt.bfloat16
    fp8 = mybir.dt.float8e4
    AF = mybir.ActivationFunctionType
    OP = mybir.AluOpType
    RSTD_POOL = bool(int(os.environ.get("RSTD_POOL", "1")))

    nc = bacc.Bacc("TRN2", target_bir_lowering=False, debug=False,
                   num_devices=NCORES)

    offs, total = _blob_layout(NROW_L)
    blob_d = nc.dram_tensor("blob", [total], bf16, kind="ExternalInput")
    out_d = nc.dram_tensor("opart", [128, 514], f32, kind="ExternalOutput")

    def bslice(key, idx=None):
        off, n, shape = offs[key]
        if idx is not None:
            per = shape[-2] * shape[-1]
            off = off + idx * per
            n = per
            shape = shape[-2:]
        ap = blob_d[off:off + n]
        return ap.rearrange("(p n) -> p n", p=shape[0])

    with tile.TileContext(nc) as tc:
        with (
            tc.tile_pool(name="const", bufs=1) as cpool,
            tc.tile_pool(name="io", bufs=int(os.environ.get("IO_BUFS", "10"))) as iopool,
            tc.tile_pool(name="work", bufs=int(os.environ.get("WK_BUFS", "3"))) as wpool,
            tc.tile_pool(name="attw", bufs=int(os.environ.get("AT_BUFS", "3"))) as apool,
            tc.tile_pool(name="mlppsum", bufs=int(os.environ.get("PS_MLP", "4")), space="PSUM") as mlpp,
            tc.tile_pool(name="svpsum", bufs=int(os.environ.get("PS_SV", "2")), space="PSUM") as svp,
            tc.tile_pool(name="opsum", bufs=1, space="PSUM") as opool,
        ):
            # ---- load constants ----
            ajT0 = cpool.tile([128, NB], bf16)
            ajT1 = cpool.tile([128, NB], bf16)
            nc.sync.dma_start(ajT0[:], bslice("ajT", 0))
            nc.sync.dma_start(ajT1[:], bslice("ajT", 1))
            ajT = [ajT0, ajT1]
            w2_s = cpool.tile([128, 512], bf16)
            g_s = cpool.tile([128, 512], bf16)
            wsv_s = cpool.tile([128, 1024], bf16)
            for t, k in ((w2_s, "w2"), (g_s, "g"), (wsv_s, "wsv")):
                nc.sync.dma_start(t[:], bslice(k))
            ones_s = cpool.tile([128, 1], bf16)
            id_s = cpool.tile([128, 128], bf16)
            nc.sync.dma_start(ones_s[:], bslice("onesw"))
            nc.sync.dma_start(id_s[:], bslice("ident"))

            # ---- persistent attention accumulators ----
            o_ps0 = opool.tile([128, 257], f32)
            o_ps1 = opool.tile([128, 257], f32)
            o_ps = [o_ps0, o_ps1]

            NIT = repeat * NROW_L
            NEWTON = int(os.environ.get("NEWTON", "2"))
            AT = int(os.environ.get("AT_BUFS", "3"))

            # pre-initialized ones columns (cycled; WAR deps keep it safe)
            v01s, v2s = [], []
            for i in range(AT):
                v01 = apool.tile([128, 514], bf16, tag="v01", name=f"v01_{i}")
                v2 = apool.tile([128, 257], bf16, tag="v2", name=f"v2_{i}")
                nc.gpsimd.memset(v01[:, 256:257], 1.0)
                nc.gpsimd.memset(v01[:, 513:514], 1.0)
                nc.gpsimd.memset(v2[:, 256:257], 1.0)
                v01s.append(v01)
                v2s.append(v2)

            def phase_dma(it):
                r = it % NROW_L
                uid = f"i{it}"
                rho_t = iopool.tile([3, NB], bf16, tag="rho", name=f"rh{uid}")
                nc.sync.dma_start(rho_t[:], bslice("rho", r))
                lA_t = iopool.tile([3, D], bf16, tag="lA", name=f"lA{uid}")
                nc.sync.dma_start(lA_t[:], bslice("lA", r))
                return {"rho": rho_t, "lA": lA_t, "it": it, "uid": uid}

            def psum_pair(tile1024):
                """[128,2,384] view of the two 384-wide chunks at 0 and 512
                of a 2-bank PSUM tile (matmul outputs stay within a bank)."""
                return tile1024[:].rearrange("p (j t) -> p j t",
                                             j=2)[:, :, 0:NB]

            def packed_pair(tile768):
                return tile768[:].rearrange("p (j t) -> p j t", j=2)

            def phase_enc1(ctx):
                uid = ctx["uid"]
                hpre = [mlpp.tile([128, NB], f32, tag="mlp",
                                  name=f"hp{uid}_{c}") for c in range(2)]
                for c in range(2):
                    nc.tensor.matmul(hpre[c][:],
                                     ctx["lA"][:, c * 128:(c + 1) * 128],
                                     ctx["rho"][:], start=True, stop=False)
                    nc.tensor.matmul(hpre[c][:], id_s[:], ajT[c][:],
                                     start=False, stop=True)
                t1 = wpool.tile([128, 768], bf16, tag="tanh", name=f"t1{uid}")
                hh = wpool.tile([128, 768], bf16, tag="hh", name=f"hh{uid}")
                for c in range(2):
                    sl = slice(c * NB, (c + 1) * NB)
                    nc.scalar.activation(t1[:, sl], hpre[c][:], AF.Tanh,
                                         scale=0.5)
                    nc.vector.scalar_tensor_tensor(hh[:, sl], t1[:, sl],
                                                   1.0, hpre[c][:],
                                                   OP.add, OP.mult)
                ctx["hh"] = hh

            def phase_enc2(ctx):
                uid = ctx["uid"]
                hh = ctx["hh"]
                l2p = [mlpp.tile([128, NB], f32, tag="mlp",
                                 name=f"l2{uid}_{c}") for c in range(2)]
                for oc in range(2):
                    for fc in range(2):
                        nc.tensor.matmul(
                            l2p[oc][:],
                            w2_s[:, (oc * 2 + fc) * 128:
                                 (oc * 2 + fc + 1) * 128],
                            hh[:, fc * NB:(fc + 1) * NB],
                            start=(fc == 0), stop=(fc == 1))
                t2 = wpool.tile([128, 768], bf16, tag="tanh", name=f"t2{uid}")
                th = wpool.tile([128, 768], bf16, tag="th", name=f"th{uid}")
                for c in range(2):
                    sl = slice(c * NB, (c + 1) * NB)
                    nc.scalar.activation(t2[:, sl], l2p[c][:], AF.Tanh,
                                         scale=0.5)
                    nc.vector.scalar_tensor_tensor(th[:, sl], t2[:, sl],
                                                   1.0, l2p[c][:],
                                                   OP.add, OP.mult)
                ctx["th"] = th

            def phase_gram(ctx):
                """E[x^2] per token via the Gram quadratic form
                th^T (w3 w3^T) th, reduced across features by tiny
                ones-matmuls."""
                uid = ctx["uid"]
                th = ctx["th"]
                y = [mlpp.tile([128, NB], f32, tag="mlp",
                               name=f"y{uid}_{c}") for c in range(2)]
                for oc in range(2):
                    for fc in range(2):
                        nc.tensor.matmul(
                            y[oc][:],
                            g_s[:, (oc * 2 + fc) * 128:
                                (oc * 2 + fc + 1) * 128],
                            th[:, fc * NB:(fc + 1) * NB],
                            start=(fc == 0), stop=(fc == 1))
                z = wpool.tile([128, 768], bf16, tag="z", name=f"z{uid}")
                nc.vector.tensor_tensor(z[:, 0:NB], th[:, 0:NB], y[0][:],
                                        OP.mult)
                nc.vector.tensor_tensor(z[:, NB:2 * NB], th[:, NB:2 * NB],
                                        y[1][:], OP.mult)
                e2col = svp.tile([128, 4], f32, tag="sv", name=f"ec{uid}")
                for sb in range(3):
                    for c in range(2):
                        nc.tensor.matmul(
                            e2col[:, sb:sb + 1],
                            z[:, c * NB + sb * 128: c * NB + sb * 128 + 128],
                            ones_s[:], start=(c == 0), stop=(c == 1))

                # rstd = 1/sqrt(e2 + eps): quake seed + Newton.  Pool only
                # supports TensorTensor (not TensorScalarPtr), so the
                # tensor_mul steps go to the otherwise-idle gpsimd engine
                # and the scalar-immediate steps stay on DVE.
                meng = nc.gpsimd if RSTD_POOL else nc.vector
                scr = wpool.tile([128, 16], f32, tag="scr", name=f"q{uid}")
                v = scr[:, 0:3]
                yq = scr[:, 4:7]
                ta = scr[:, 8:11]
                tb = scr[:, 12:15]
                nc.vector.tensor_scalar(v, e2col[:, 0:3], EPS, None, OP.add)
                v_u = v.bitcast(mybir.dt.uint32)
                y_u = yq.bitcast(mybir.dt.uint32)
                nc.vector.tensor_scalar(y_u, v_u, 1, None,
                                        OP.logical_shift_right)
                nc.vector.tensor_scalar(y_u, y_u, 0xA0C8A620, None, OP.add)
                nc.vector.tensor_scalar(y_u, y_u, 0xFFFFFFFF, None,
                                        OP.bitwise_xor)
                rstd = wpool.tile([128, 3], f32, tag="rstd", name=f"rs{uid}")
                for nit in range(NEWTON):
                    dst = yq if nit < NEWTON - 1 else rstd[:, 0:3]
                    meng.tensor_mul(ta, yq, yq)
                    meng.tensor_mul(tb, ta, v)
                    nc.vector.tensor_scalar(ta, tb, -0.5, 1.5,
                                            OP.mult, OP.add)
                    meng.tensor_mul(dst, yq, ta)
                ctx["rstd"] = rstd

            def phase_att(ctx):
                uid = ctx["uid"]
                it = ctx["it"]
                th = ctx["th"]
                rstd = ctx["rstd"]
                pT01 = apool.tile([128, 512], bf16, tag="pT01",
                                  name=f"p{uid}")
                pT2 = apool.tile([128, 256], bf16, tag="pT2", name=f"p2{uid}")
                v01 = v01s[it % AT]
                v2 = v2s[it % AT]
                pdst = [pT01[:, 0:256], pT01[:, 256:512], pT2[:, 0:256]]
                vdst = [v01[:, 0:256], v01[:, 257:513], v2[:, 0:256]]
                for sb in range(3):
                    svt = svp.tile([128, 512], f32, tag="sv",
                                   name=f"sv{uid}_{sb}")
                    for fc in range(2):
                        nc.tensor.matmul(
                            svt[:],
                            th[:, fc * NB + sb * 128:
                               fc * NB + sb * 128 + 128],
                            wsv_s[:, fc * 512:(fc + 1) * 512],
                            start=(fc == 0), stop=(fc == 1))
                    nc.scalar.activation(pdst[sb], svt[:, 0:256], AF.Exp,
                                         scale=rstd[:, sb:sb + 1])
                    # v-scale: balanced between DVE and ACT (identity+scale)
                    va = int(os.environ.get("V_ACT", "2"))
                    on_act = (sb == 1) or (sb == 2 and (va == 3 or (
                        va == 2 and it % 2 == 0))) or (sb == 0 and va == 4)
                    if on_act:
                        nc.scalar.activation(vdst[sb], svt[:, 256:512],
                                             AF.Identity,
                                             scale=rstd[:, sb:sb + 1])
                    else:
                        nc.vector.tensor_scalar(vdst[sb], svt[:, 256:512],
                                                rstd[:, sb:sb + 1], None,
                                                OP.mult)
                ctx["pT"] = (pT01, pT2)
                ctx["v"] = (v01, v2)

            def phase_O(ctx):
                it = ctx["it"]
                pT01, pT2 = ctx["pT"]
                v01, v2 = ctx["v"]
                psrc = [(pT01, 0), (pT01, 256), (pT2, 0)]
                vsrc = [(v01, 0), (v01, 257), (v2, 0)]
                for sb in range(3):
                    pt, po = psrc[sb]
                    vt, vo = vsrc[sb]
                    for oc in range(2):
                        nc.tensor.matmul(
                            o_ps[oc][:, 0:257],
                            pt[:, po + oc * 128:po + (oc + 1) * 128],
                            vt[:, vo:vo + 257],
                            start=(it == 0 and sb == 0),
                            stop=(it == NIT - 1 and sb == 2))

            # --- 3-deep software pipeline over rows ---
            ctxs = {}
            ctxs[0] = phase_dma(0)
            ctxs[1] = phase_dma(1)
            for k in range(NIT + 2):
                if k + 2 < NIT:
                    ctxs[k + 2] = phase_dma(k + 2)
                if k < NIT:
                    phase_enc1(ctxs[k])
                if 0 <= k - 1 < NIT:
                    phase_gram(ctxs[k - 1])
                if k < NIT:
                    phase_enc2(ctxs[k])
                if 0 <= k - 2:
                    phase_att(ctxs[k - 2])
                    phase_O(ctxs[k - 2])
                    del ctxs[k - 2]

            # ---- write out partial results ----
            ostage = cpool.tile([128, 514], f32)
            nc.vector.tensor_copy(ostage[:, 0:257], o_ps0[:])
            nc.vector.tensor_copy(ostage[:, 257:514], o_ps1[:])
            nc.sync.dma_start(out_d[:], ostage[:])

    nc.compile()
    return nc


def _get_program():
    global _PROGRAM
    if _PROGRAM is None:
        _PROGRAM = _build_program()
    return _PROGRAM


def build_in_maps(inp):
    f = np.float32

    # ---------------- host precompute (tiny, O(nb*D^2)) ----------------
    Z = inp["Z"].astype(np.int64)
    ang_l = inp["ang_l"].astype(np.int64)
    m_sh = np.clip(inp["mag_m"].astype(np.int64) + MAX_L, 0, 2 * MAX_L)
    orb_in = np.concatenate([inp["elem_emb"][Z], inp["l_emb"][ang_l],
                             inp["m_emb"][m_sh]], axis=-1).astype(f)
    orb = (_np_silu(orb_in @ inp["proj_w1"] + inp["proj_b1"])
           @ inp["proj_w2"] + inp["proj_b2"]).astype(f)

    enc_w1 = inp["enc_w1"].astype(f)
    a_i = orb @ enc_w1[:128]
    a_j = orb @ enc_w1[128:256]
    w_r = enc_w1[256]
    w_im = enc_w1[257]
    a_ib = a_i + inp["enc_b1"].astype(f)

    if not (np.all(inp["enc_b2"] == 0) and np.all(inp["enc_b3"] == 0)):
        raise NotImplementedError("nonzero enc_b2/enc_b3 not supported")

    lnw = inp["ln_kv_w"].astype(f)
    wk_p = lnw[:, None] * inp["wk"].astype(f)
    wv_p = lnw[:, None] * inp["wv"].astype(f)

    qn = _np_layernorm(inp["query_tokens"].astype(f), inp["ln_q_w"].astype(f),
                       inp["ln_q_b"].astype(f))
    Q = (qn @ inp["wq"].astype(f) + inp["bq"].astype(f)).reshape(TQ, H, DH)

    WQ = np.zeros((D, D), f)
    for h in range(H):
        WQ[:, h * DH:(h + 1) * DH] = (wk_p[:, h * DH:(h + 1) * DH]
                                      @ Q[:, h, :].T) / np.sqrt(DH)

    w2p = 0.5 * inp["enc_w2"].astype(f)
    # centering w3's output columns makes kv_pre exactly zero-mean over
    # features, which is what layernorm subtracts -- mu pipeline vanishes.
    w3p = 0.5 * inp["enc_w3"].astype(f)
    w3p = w3p - w3p.mean(axis=1, keepdims=True)

    # w3 is linear before the layernorm: fold it into the attention
    # projections and compute E[x^2] via the Gram quadratic form.
    W_SV = w3p @ np.concatenate([WQ, wv_p], axis=1)     # (256, 512)
    G = (w3p @ w3p.T).astype(f)                          # (256, 256)

    def chunk4(w):  # [256, 256] -> [128, 512] stationary chunks (oc*2+fc)
        out = np.empty((128, 512), f)
        for oc in range(2):
            for fc in range(2):
                out[:, (oc * 2 + fc) * 128:(oc * 2 + fc + 1) * 128] = \
                    w[fc * 128:(fc + 1) * 128, oc * 128:(oc + 1) * 128]
        return out

    rho_r = inp["rho_real"].astype(f).reshape(NROW_TOTAL, NB)
    rho_i = inp["rho_imag"].astype(f).reshape(NROW_TOTAL, NB)

    common = {
        "ajT": np.ascontiguousarray(a_j.T.reshape(2, 128, NB)),
        "w2": chunk4(w2p),
        "g": chunk4(G),
        "wsv": np.concatenate([W_SV[0:128, :], W_SV[128:256, :]],
                              axis=1),
        "onesw": np.full((128, 1), 1.0 / D, f),
        "ident": np.eye(128, dtype=f),
    }

    in_maps = []
    for c in range(NCORES):
        rows = slice(c * NROW, (c + 1) * NROW)
        g = np.arange(c * NROW, (c + 1) * NROW)
        i_idx = g % NB
        rho = np.empty((NROW, 3, NB), f)
        rho[:, 0, :] = rho_r[rows]
        rho[:, 1, :] = rho_i[rows]
        rho[:, 2, :] = 1.0
        lA = np.empty((NROW, 3, D), f)
        lA[:, 0, :] = w_r
        lA[:, 1, :] = w_im
        lA[:, 2, :] = a_ib[i_idx]
        parts = dict(common)
        parts["rho"] = rho
        parts["lA"] = lA
        in_maps.append({"blob": _pack_blob(parts, NROW)})
    return in_maps


def combine_results(inp, core_results):
    f = np.float32
    num = np.zeros((H, TQ, DH), np.float64)
    den = np.zeros((H, TQ), np.float64)
    for c in range(NCORES):
        arr = np.asarray(core_results[c]["opart"], f)
        for h in range(H):
            oc, hrel = divmod(h, 4)
            blk = arr[:, oc * 257:(oc + 1) * 257]
            rows_ = slice(hrel * TQ, (hrel + 1) * TQ)
            num[h] += blk[rows_, h * DH:(h + 1) * DH]
            den[h] += blk[rows_, 256]

    lnb = inp["ln_kv_b"].astype(f)
    ctx = np.empty((TQ, D), f)
    for h in range(H):
        ctx[:, h * DH:(h + 1) * DH] = (num[h] / den[h][:, None]).astype(f)
    cv = inp["wv"].astype(f).T @ lnb + inp["bv"].astype(f)
    ctx = ctx + cv

    attended = ctx @ inp["wo"].astype(f) + inp["bo"].astype(f)
    y = (_np_silu(attended @ inp["out_w1"].astype(f) + inp["out_b1"].astype(f))
         @ inp["out_w2"].astype(f) + inp["out_b2"].astype(f))
    return y.astype(np.float32)


def kernel(**inputs):
    global LAST_EXEC_NS, LAST_RESULTS
    inp = {k: np.asarray(v) for k, v in inputs.items()}
    in_maps = build_in_maps(inp)

    # ---------------- run on the 8 NeuronCores ----------------
    from concourse.bass_utils import run_bass_kernel_spmd

    nc = _get_program()
    trace = bool(int(os.environ.get("BASS_KERNEL_TRACE", "0")))
    try:
        res = run_bass_kernel_spmd(nc, in_maps, list(range(NCORES)),
                                   trace=trace)
    except Exception:
        if not trace:
            raise
        res = run_bass_kernel_spmd(nc, in_maps, list(range(NCORES)),
                                   trace=False)
    LAST_EXEC_NS = res.exec_time_ns
    LAST_RESULTS = res
    return combine_results(inp, res.results)


# revision 25
# speedup vs baseline: 2.3958x; 2.3958x over previous
"""Trainium2 Bass kernel for nn_DensityEdgeProjection.

Strategy (sharding_hint): the ns*nb*nb density-element (KV token) axis is
sharded over 8 cores as 768 (spin, i) rows of 384 j-tokens each -> 96 rows
per core.  Each core encodes its token slice and produces partial attention
numerators/denominators (no max -- scores are provably tiny), which the host
combines (flash-attention style) and pushes through the small output MLP.

Key algebraic restructure vs the v1 kernel: encoder layer 3 is linear
(no activation before the layernorm), so its weight w3 is folded on the
host into the attention projections (W_SV = w3p @ [WQ | wv_p]) and the
layernorm statistic E[x^2] is computed as the quadratic form
that^T (w3p w3p^T/D) that.  This removes the L3 matmul output
materialization (PSUM->SBUF copy) and the x^2 squaring pass.

Device layout: activations are feature-major (features on partitions,
tokens on the free axis).  All matmuls bf16 with fp32 PSUM accumulation.
S and V projections are fused into one 512-wide matmul per token block
(exactly one PSUM bank).
"""

import os
import sys
import numpy as np

sys.path.insert(0, "/opt/trn_rl_repo")

from ml_dtypes import bfloat16

NB = 384
NS = 2
D = 256
TQ = 32
H = 8
DH = 32
MAX_L = 2
NCORES = 8
NROW_TOTAL = NS * NB          # 768 (s, i) rows
NROW = NROW_TOTAL // NCORES   # 96 rows per core
EPS = 1e-5

LAST_EXEC_NS = None
LAST_RESULTS = None

_PROGRAM = None


def _np_silu(x):
    return x / (1.0 + np.exp(-x))


def _np_layernorm(x, w, b):
    mu = x.mean(-1, keepdims=True)
    var = x.var(-1, keepdims=True)
    return (x - mu) / np.sqrt(var + EPS) * w + b


S3 = 16.0   # fp8 range scale for the folded w3 projection


def _blob_layout(nrow):
    """element offsets (bf16) for each packed section, 512-aligned."""
    sections = [
        ("rho", (nrow, 3, NB)),
        ("lA", (nrow, 3, D)),
        ("ajT", (2, 128, NB)),
        ("w2", (128, 512)),
        ("g", (128, 512)),
        ("wsv", (128, 1024)),
        ("onesw", (128, 1)),
        ("ident", (128, 128)),
    ]
    offs = {}
    off = 0
    for k, shp in sections:
        n = int(np.prod(shp))
        offs[k] = (off, n, shp)
        off += (n + 511) // 512 * 512
    return offs, off


def _pack_blob(parts, nrow):
    offs, total = _blob_layout(nrow)
    blob = np.zeros(total, bfloat16)
    for k, arr in parts.items():
        off, n, shp = offs[k]
        assert tuple(arr.shape) == tuple(shp), (k, arr.shape, shp)
        blob[off:off + n] = arr.astype(bfloat16).ravel()
    return blob


def _build_program(nrow=NROW, repeat=1):
    """Build the (single, shared across cores) Bass/Tile program."""
    NROW_L = nrow
    import concourse.bass as bass
    import concourse.bacc as bacc
    import concourse.tile as tile
    from concourse import mybir

    f32 = mybir.dt.float32
    bf16 = mybir.dt.bfloat16
    fp8 = mybir.dt.float8e4
    AF = mybir.ActivationFunctionType
    OP = mybir.AluOpType
    RSTD_POOL = bool(int(os.environ.get("RSTD_POOL", "1")))

    nc = bacc.Bacc("TRN2", target_bir_lowering=False, debug=False,
                   num_devices=NCORES)

    offs, total = _blob_layout(NROW_L)
    blob_d = nc.dram_tensor("blob", [total], bf16, kind="ExternalInput")
    out_d = nc.dram_tensor("opart", [128, 514], f32, kind="ExternalOutput")

    def bslice(key, idx=None):
        off, n, shape = offs[key]
        if idx is not None:
            per = shape[-2] * shape[-1]
            off = off + idx * per
            n = per
            shape = shape[-2:]
        ap = blob_d[off:off + n]
        return ap.rearrange("(p n) -> p n", p=shape[0])

    with tile.TileContext(nc) as tc:
        with (
            tc.tile_pool(name="const", bufs=1) as cpool,
            tc.tile_pool(name="io", bufs=int(os.environ.get("IO_BUFS", "10"))) as iopool,
            tc.tile_pool(name="work", bufs=int(os.environ.get("WK_BUFS", "3"))) as wpool,
            tc.tile_pool(name="attw", bufs=int(os.environ.get("AT_BUFS", "3"))) as apool,
            tc.tile_pool(name="mlppsum", bufs=int(os.environ.get("PS_MLP", "4")), space="PSUM") as mlpp,
            tc.tile_pool(name="svpsum", bufs=int(os.environ.get("PS_SV", "2")), space="PSUM") as svp,
            tc.tile_pool(name="opsum", bufs=1, space="PSUM") as opool,
        ):
            # ---- load constants ----
            ajT0 = cpool.tile([128, NB], bf16)
            ajT1 = cpool.tile([128, NB], bf16)
            nc.sync.dma_start(ajT0[:], bslice("ajT", 0))
            nc.sync.dma_start(ajT1[:], bslice("ajT", 1))
            ajT = [ajT0, ajT1]
            w2_s = cpool.tile([128, 512], bf16)
            g_s = cpool.tile([128, 512], bf16)
            wsv_s = cpool.tile([128, 1024], bf16)
            for t, k in ((w2_s, "w2"), (g_s, "g"), (wsv_s, "wsv")):
                nc.sync.dma_start(t[:], bslice(k))
            ones_s = cpool.tile([128, 1], bf16)
            id_s = cpool.tile([128, 128], bf16)
            nc.sync.dma_start(ones_s[:], bslice("onesw"))
            nc.sync.dma_start(id_s[:], bslice("ident"))

            # ---- persistent attention accumulators ----
            o_ps0 = opool.tile([128, 257], f32)
            o_ps1 = opool.tile([128, 257], f32)
            o_ps = [o_ps0, o_ps1]

            NIT = repeat * NROW_L
            NEWTON = int(os.environ.get("NEWTON", "2"))
            AT = int(os.environ.get("AT_BUFS", "3"))

            # pre-initialized ones columns (cycled; WAR deps keep it safe)
            v01s, v2s = [], []
            for i in range(AT):
                v01 = apool.tile([128, 514], bf16, tag="v01", name=f"v01_{i}")
                v2 = apool.tile([128, 257], bf16, tag="v2", name=f"v2_{i}")
                nc.gpsimd.memset(v01[:, 256:257], 1.0)
                nc.gpsimd.memset(v01[:, 513:514], 1.0)
                nc.gpsimd.memset(v2[:, 256:257], 1.0)
                v01s.append(v01)
                v2s.append(v2)

            def phase_dma(it):
                r = it % NROW_L
                uid = f"i{it}"
                rho_t = iopool.tile([3, NB], bf16, tag="rho", name=f"rh{uid}")
                nc.sync.dma_start(rho_t[:], bslice("rho", r))
                lA_t = iopool.tile([3, D], bf16, tag="lA", name=f"lA{uid}")
                nc.sync.dma_start(lA_t[:], bslice("lA", r))
                return {"rho": rho_t, "lA": lA_t, "it": it, "uid": uid}

            def psum_pair(tile1024):
                """[128,2,384] view of the two 384-wide chunks at 0 and 512
                of a 2-bank PSUM tile (matmul outputs stay within a bank)."""
                return tile1024[:].rearrange("p (j t) -> p j t",
                                             j=2)[:, :, 0:NB]

            def packed_pair(tile768):
                return tile768[:].rearrange("p (j t) -> p j t", j=2)

            def phase_enc1(ctx):
                uid = ctx["uid"]
                hpre = [mlpp.tile([128, NB], f32, tag="mlp",
                                  name=f"hp{uid}_{c}") for c in range(2)]
                for c in range(2):
                    nc.tensor.matmul(hpre[c][:],
                                     ctx["lA"][:, c * 128:(c + 1) * 128],
                                     ctx["rho"][:], start=True, stop=False)
                    nc.tensor.matmul(hpre[c][:], id_s[:], ajT[c][:],
                                     start=False, stop=True)
                t1 = wpool.tile([128, 768], bf16, tag="tanh", name=f"t1{uid}")
                hh = wpool.tile([128, 768], bf16, tag="hh", name=f"hh{uid}")
                for c in range(2):
                    sl = slice(c * NB, (c + 1) * NB)
                    nc.scalar.activation(t1[:, sl], hpre[c][:], AF.Tanh,
                                         scale=0.5)
                    nc.vector.scalar_tensor_tensor(hh[:, sl], t1[:, sl],
                                                   1.0, hpre[c][:],
                                                   OP.add, OP.mult)
                ctx["hh"] = hh

            def phase_enc2(ctx):
                uid = ctx["uid"]
                hh = ctx["hh"]
                l2p = [mlpp.tile([128, NB], f32, tag="mlp",
                                 name=f"l2{uid}_{c}") for c in range(2)]
                for oc in range(2):
                    for fc in range(2):
                        nc.tensor.matmul(
                            l2p[oc][:],
                            w2_s[:, (oc * 2 + fc) * 128:
                                 (oc * 2 + fc + 1) * 128],
                            hh[:, fc * NB:(fc + 1) * NB],
                            start=(fc == 0), stop=(fc == 1))
                t2 = wpool.tile([128, 768], bf16, tag="tanh", name=f"t2{uid}")
                th = wpool.tile([128, 768], bf16, tag="th", name=f"th{uid}")
                for c in range(2):
                    sl = slice(c * NB, (c + 1) * NB)
                    nc.scalar.activation(t2[:, sl], l2p[c][:], AF.Tanh,
                                         scale=0.5)
                    nc.vector.scalar_tensor_tensor(th[:, sl], t2[:, sl],
                                                   1.0, l2p[c][:],
                                                   OP.add, OP.mult)
                ctx["th"] = th

            def phase_gram(ctx):
                """E[x^2] per token via the Gram quadratic form
                th^T (w3 w3^T) th, reduced across features by tiny
                ones-matmuls."""
                uid = ctx["uid"]
                th = ctx["th"]
                y = [mlpp.tile([128, NB], f32, tag="mlp",
                               name=f"y{uid}_{c}") for c in range(2)]
                for oc in range(2):
                    for fc in range(2):
                        nc.tensor.matmul(
                            y[oc][:],
                            g_s[:, (oc * 2 + fc) * 128:
                                (oc * 2 + fc + 1) * 128],
                            th[:, fc * NB:(fc + 1) * NB],
                            start=(fc == 0), stop=(fc == 1))
                z = wpool.tile([128, 768], bf16, tag="z", name=f"z{uid}")
                nc.vector.tensor_tensor(z[:, 0:NB], th[:, 0:NB], y[0][:],
                                        OP.mult)
                nc.vector.tensor_tensor(z[:, NB:2 * NB], th[:, NB:2 * NB],
                                        y[1][:], OP.mult)
                e2col = svp.tile([128, 4], f32, tag="sv", name=f"ec{uid}")
                for sb in range(3):
                    for c in range(2):
                        nc.tensor.matmul(
                            e2col[:, sb:sb + 1],
                            z[:, c * NB + sb * 128: c * NB + sb * 128 + 128],
                            ones_s[:], start=(c == 0), stop=(c == 1))

                # rstd = 1/sqrt(e2 + eps): quake seed + Newton.  Pool only
                # supports TensorTensor (not TensorScalarPtr), so the
                # tensor_mul steps go to the otherwise-idle gpsimd engine
                # and the scalar-immediate steps stay on DVE.
                meng = nc.gpsimd if RSTD_POOL else nc.vector
                scr = wpool.tile([128, 16], f32, tag="scr", name=f"q{uid}")
                v = scr[:, 0:3]
                yq = scr[:, 4:7]
                ta = scr[:, 8:11]
                tb = scr[:, 12:15]
                nc.vector.tensor_scalar(v, e2col[:, 0:3], EPS, None, OP.add)
                v_u = v.bitcast(mybir.dt.uint32)
                y_u = yq.bitcast(mybir.dt.uint32)
                nc.vector.tensor_scalar(y_u, v_u, 1, None,
                                        OP.logical_shift_right)
                nc.vector.tensor_scalar(y_u, y_u, 0xA0C8A620, None, OP.add)
                nc.vector.tensor_scalar(y_u, y_u, 0xFFFFFFFF, None,
                                        OP.bitwise_xor)
                rstd = wpool.tile([128, 3], f32, tag="rstd", name=f"rs{uid}")
                for nit in range(NEWTON):
                    dst = yq if nit < NEWTON - 1 else rstd[:, 0:3]
                    meng.tensor_mul(ta, yq, yq)
                    meng.tensor_mul(tb, ta, v)
                    nc.vector.tensor_scalar(ta, tb, -0.5, 1.5,
                                            OP.mult, OP.add)
                    meng.tensor_mul(dst, yq, ta)
                ctx["rstd"] = rstd

            def phase_att(ctx):
                uid = ctx["uid"]
                it = ctx["it"]
                th = ctx["th"]
                rstd = ctx["rstd"]
                pT01 = apool.tile([128, 512], bf16, tag="pT01",
                                  name=f"p{uid}")
                pT2 = apool.tile([128, 256], bf16, tag="pT2", name=f"p2{uid}")
                v01 = v01s[it % AT]
                v2 = v2s[it % AT]
                pdst = [pT01[:, 0:256], pT01[:, 256:512], pT2[:, 0:256]]
                vdst = [v01[:, 0:256], v01[:, 257:513], v2[:, 0:256]]
                for sb in range(3):
                    svt = svp.tile([128, 512], f32, tag="sv",
                                   name=f"sv{uid}_{sb}")
                    for fc in range(2):
                        nc.tensor.matmul(
                            svt[:],
                            th[:, fc * NB + sb * 128:
                               fc * NB + sb * 128 + 128],
                            wsv_s[:, fc * 512:(fc + 1) * 512],
                            start=(fc == 0), stop=(fc == 1))
                    nc.scalar.activation(pdst[sb], svt[:, 0:256], AF.Exp,
                                         scale=rstd[:, sb:sb + 1])
                    # v-scale: balanced between DVE and ACT (identity+scale)
                    va = int(os.environ.get("V_ACT", "2"))
                    on_act = (sb == 1) or (sb == 2 and (va == 3 or (
                        va == 2 and it % 2 == 0))) or (sb == 0 and va == 4)
                    if on_act:
                        nc.scalar.activation(vdst[sb], svt[:, 256:512],
                                             AF.Identity,
                                             scale=rstd[:, sb:sb + 1])
                    else:
                        nc.vector.tensor_scalar(vdst[sb], svt[:, 256:512],
                                                rstd[:, sb:sb + 1], None,
                                                OP.mult)
                ctx["pT"] = (pT01, pT2)
                ctx["v"] = (v01, v2)

            def phase_O(ctx):
                it = ctx["it"]
                pT01, pT2 = ctx["pT"]
                v01, v2 = ctx["v"]
                psrc = [(pT01, 0), (pT01, 256), (pT2, 0)]
                vsrc = [(v01, 0), (v01, 257), (v2, 0)]
                for sb in range(3):
                    pt, po = psrc[sb]
                    vt, vo = vsrc[sb]
                    for oc in range(2):
                        nc.tensor.matmul(
                            o_ps[oc][:, 0:257],
                            pt[:, po + oc * 128:po + (oc + 1) * 128],
                            vt[:, vo:vo + 257],
                            start=(it == 0 and sb == 0),
                            stop=(it == NIT - 1 and sb == 2))

            # --- 3-deep software pipeline over rows ---
            ctxs = {}
            ctxs[0] = phase_dma(0)
            ctxs[1] = phase_dma(1)
            PIPE = os.environ.get("PIPE", "a")
            for k in range(NIT + 2):
                if k + 2 < NIT:
                    ctxs[k + 2] = phase_dma(k + 2)
                if PIPE == "b" and 0 <= k - 1 < NIT:
                    phase_gram(ctxs[k - 1])
                if k < NIT:
                    phase_enc1(ctxs[k])
                if PIPE == "a" and 0 <= k - 1 < NIT:
                    phase_gram(ctxs[k - 1])
                if PIPE == "c" and 0 <= k - 2:
                    phase_att(ctxs[k - 2])
                if k < NIT:
                    phase_enc2(ctxs[k])
                if PIPE != "c" and 0 <= k - 2:
                    phase_att(ctxs[k - 2])
                if 0 <= k - 2:
                    phase_O(ctxs[k - 2])
                    del ctxs[k - 2]

            # ---- write out partial results ----
            ostage = cpool.tile([128, 514], f32)
            nc.vector.tensor_copy(ostage[:, 0:257], o_ps0[:])
            nc.vector.tensor_copy(ostage[:, 257:514], o_ps1[:])
            nc.sync.dma_start(out_d[:], ostage[:])

    nc.compile()
    return nc


def _get_program():
    global _PROGRAM
    if _PROGRAM is None:
        _PROGRAM = _build_program()
    return _PROGRAM


def build_in_maps(inp):
    f = np.float32

    # ---------------- host precompute (tiny, O(nb*D^2)) ----------------
    Z = inp["Z"].astype(np.int64)
    ang_l = inp["ang_l"].astype(np.int64)
    m_sh = np.clip(inp["mag_m"].astype(np.int64) + MAX_L, 0, 2 * MAX_L)
    orb_in = np.concatenate([inp["elem_emb"][Z], inp["l_emb"][ang_l],
                             inp["m_emb"][m_sh]], axis=-1).astype(f)
    orb = (_np_silu(orb_in @ inp["proj_w1"] + inp["proj_b1"])
           @ inp["proj_w2"] + inp["proj_b2"]).astype(f)

    enc_w1 = inp["enc_w1"].astype(f)
    a_i = orb @ enc_w1[:128]
    a_j = orb @ enc_w1[128:256]
    w_r = enc_w1[256]
    w_im = enc_w1[257]
    a_ib = a_i + inp["enc_b1"].astype(f)

    if not (np.all(inp["enc_b2"] == 0) and np.all(inp["enc_b3"] == 0)):
        raise NotImplementedError("nonzero enc_b2/enc_b3 not supported")

    lnw = inp["ln_kv_w"].astype(f)
    wk_p = lnw[:, None] * inp["wk"].astype(f)
    wv_p = lnw[:, None] * inp["wv"].astype(f)

    qn = _np_layernorm(inp["query_tokens"].astype(f), inp["ln_q_w"].astype(f),
                       inp["ln_q_b"].astype(f))
    Q = (qn @ inp["wq"].astype(f) + inp["bq"].astype(f)).reshape(TQ, H, DH)

    WQ = np.zeros((D, D), f)
    for h in range(H):
        WQ[:, h * DH:(h + 1) * DH] = (wk_p[:, h * DH:(h + 1) * DH]
                                      @ Q[:, h, :].T) / np.sqrt(DH)

    w2p = 0.5 * inp["enc_w2"].astype(f)
    # centering w3's output columns makes kv_pre exactly zero-mean over
    # features, which is what layernorm subtracts -- mu pipeline vanishes.
    w3p = 0.5 * inp["enc_w3"].astype(f)
    w3p = w3p - w3p.mean(axis=1, keepdims=True)

    # w3 is linear before the layernorm: fold it into the attention
    # projections and compute E[x^2] via the Gram quadratic form.
    W_SV = w3p @ np.concatenate([WQ, wv_p], axis=1)     # (256, 512)
    G = (w3p @ w3p.T).astype(f)                          # (256, 256)

    def chunk4(w):  # [256, 256] -> [128, 512] stationary chunks (oc*2+fc)
        out = np.empty((128, 512), f)
        for oc in range(2):
            for fc in range(2):
                out[:, (oc * 2 + fc) * 128:(oc * 2 + fc + 1) * 128] = \
                    w[fc * 128:(fc + 1) * 128, oc * 128:(oc + 1) * 128]
        return out

    rho_r = inp["rho_real"].astype(f).reshape(NROW_TOTAL, NB)
    rho_i = inp["rho_imag"].astype(f).reshape(NROW_TOTAL, NB)

    common = {
        "ajT": np.ascontiguousarray(a_j.T.reshape(2, 128, NB)),
        "w2": chunk4(w2p),
        "g": chunk4(G),
        "wsv": np.concatenate([W_SV[0:128, :], W_SV[128:256, :]],
                              axis=1),
        "onesw": np.full((128, 1), 1.0 / D, f),
        "ident": np.eye(128, dtype=f),
    }

    in_maps = []
    for c in range(NCORES):
        rows = slice(c * NROW, (c + 1) * NROW)
        g = np.arange(c * NROW, (c + 1) * NROW)
        i_idx = g % NB
        rho = np.empty((NROW, 3, NB), f)
        rho[:, 0, :] = rho_r[rows]
        rho[:, 1, :] = rho_i[rows]
        rho[:, 2, :] = 1.0
        lA = np.empty((NROW, 3, D), f)
        lA[:, 0, :] = w_r
        lA[:, 1, :] = w_im
        lA[:, 2, :] = a_ib[i_idx]
        parts = dict(common)
        parts["rho"] = rho
        parts["lA"] = lA
        in_maps.append({"blob": _pack_blob(parts, NROW)})
    return in_maps


def combine_results(inp, core_results):
    f = np.float32
    num = np.zeros((H, TQ, DH), np.float64)
    den = np.zeros((H, TQ), np.float64)
    for c in range(NCORES):
        arr = np.asarray(core_results[c]["opart"], f)
        for h in range(H):
            oc, hrel = divmod(h, 4)
            blk = arr[:, oc * 257:(oc + 1) * 257]
            rows_ = slice(hrel * TQ, (hrel + 1) * TQ)
            num[h] += blk[rows_, h * DH:(h + 1) * DH]
            den[h] += blk[rows_, 256]

    lnb = inp["ln_kv_b"].astype(f)
    ctx = np.empty((TQ, D), f)
    for h in range(H):
        ctx[:, h * DH:(h + 1) * DH] = (num[h] / den[h][:, None]).astype(f)
    cv = inp["wv"].astype(f).T @ lnb + inp["bv"].astype(f)
    ctx = ctx + cv

    attended = ctx @ inp["wo"].astype(f) + inp["bo"].astype(f)
    y = (_np_silu(attended @ inp["out_w1"].astype(f) + inp["out_b1"].astype(f))
         @ inp["out_w2"].astype(f) + inp["out_b2"].astype(f))
    return y.astype(np.float32)


def kernel(**inputs):
    global LAST_EXEC_NS, LAST_RESULTS
    inp = {k: np.asarray(v) for k, v in inputs.items()}
    in_maps = build_in_maps(inp)

    # ---------------- run on the 8 NeuronCores ----------------
    from concourse.bass_utils import run_bass_kernel_spmd

    nc = _get_program()
    trace = bool(int(os.environ.get("BASS_KERNEL_TRACE", "0")))
    try:
        res = run_bass_kernel_spmd(nc, in_maps, list(range(NCORES)),
                                   trace=trace)
    except Exception:
        if not trace:
            raise
        res = run_bass_kernel_spmd(nc, in_maps, list(range(NCORES)),
                                   trace=False)
    LAST_EXEC_NS = res.exec_time_ns
    LAST_RESULTS = res
    return combine_results(inp, res.results)
